# revision 1
# baseline (speedup 1.0000x reference)
"""CombinedCSA (channel+spatial attention) Trainium2 Bass kernel.

Sharding: data-parallel over batch. 16 images / 8 cores = 2 images per core.
Weights (fc1/fc2/conv) replicated, pre-transposed host-side.

Per-core dataflow (per image, streamed in HW chunks of 16 rows):
  load chunk -> channel-max (DVE reduce) + channel-sum (ACT accum_out)
  MLP (PE matmuls + ACT relu/sigmoid) -> per-channel scale
  scale chunk in place (ACT, per-partition scale)
  spatial max over C: DVE max(half0,half1) -> PE transpose -> DVE segmented reduce
  spatial sum over C: PE matmul (x block stationary, ones moving)
  7x7 conv: 14 banded matmuls on PE (bands built host-side)
  sigmoid -> transpose -> row-collapse DMA -> gpsimd partition_broadcast
  final multiply in place (DVE / gpsimd split) -> store
"""

import os
import numpy as np
from contextlib import ExitStack

import concourse.bass as bass
import concourse.tile as tile
from concourse import bacc, mybir
from concourse._compat import with_exitstack
from concourse.bass_utils import run_bass_kernel_spmd

F32 = mybir.dt.float32
AF = mybir.ActivationFunctionType

# Problem constants (hardcoded; see spec)
B, C, H, W = 16, 256, 128, 128
HW = H * W          # 16384
R = 16              # Cr = C // 16
NCORES = 8
BLOC = B // NCORES  # 2 images per core
NH = 2              # channel halves of 128
P = 128
FCH = 2048          # hw elements per chunk (16 h-rows)
NCH = HW // FCH     # 8 chunks per image
HROWS = FCH // W    # 16 h-rows per chunk
CONVG = 2           # chunks per conv group
NBLK = FCH // P     # 16 transpose blocks per chunk

# chunk indices whose heavy elementwise ops go to gpsimd instead of DVE
# (walrus rejects TensorTensor on the Pool engine on this toolchain, so empty)
GPS_FINAL = frozenset()
GPS_COMBINE = frozenset()


@with_exitstack
def csa_kernel(ctx, tc, out_d, x_d, w1t_d, w2t_d, bands_d, ident_d,
               skip=frozenset()):
    nc = tc.nc

    # ---- pools ----
    xp = ctx.enter_context(tc.tile_pool(name="xp", bufs=19))
    xmaxp = ctx.enter_context(tc.tile_pool(name="xmaxp", bufs=2))
    bcp = ctx.enter_context(tc.tile_pool(name="bcp", bufs=1))
    rowp = ctx.enter_context(tc.tile_pool(name="rowp", bufs=1))
    stat = ctx.enter_context(tc.tile_pool(name="stat", bufs=2))
    cons = ctx.enter_context(tc.tile_pool(name="cons", bufs=1))
    tp = ctx.enter_context(tc.tile_pool(name="tp", bufs=2, space="PSUM"))
    tsp = ctx.enter_context(tc.tile_pool(name="tsp", bufs=2, space="PSUM"))
    convp = ctx.enter_context(tc.tile_pool(name="convp", bufs=2, space="PSUM"))
    atpp = ctx.enter_context(tc.tile_pool(name="atpp", bufs=1, space="PSUM"))
    mlpp = ctx.enter_context(tc.tile_pool(name="mlpp", bufs=1, space="PSUM"))

    # ---- constants / weights ----
    w1t_sb = cons.tile([P, NH * R], F32)           # [128, 32]: col block h = w_fc1.T half h
    for h in range(NH):
        nc.sync.dma_start(out=w1t_sb[:, h * R:(h + 1) * R],
                          in_=w1t_d[h * P:(h + 1) * P, :])
    w2t_sb = cons.tile([R, C], F32)                # [16, 256] = w_fc2.T
    nc.sync.dma_start(out=w2t_sb[:], in_=w2t_d[:])
    bands_sb = cons.tile([P, 14 * P], F32)         # [128, (ci, w)]
    nc.sync.dma_start(out=bands_sb[:].rearrange("p (c w) -> p c w", c=14),
                      in_=bands_d.transpose([1, 0, 2]))
    ident_sb = cons.tile([P, P], F32)
    nc.sync.dma_start(out=ident_sb[:], in_=ident_d[:])
    ones_sb = cons.tile([P, 1], F32)
    nc.vector.memset(ones_sb[:], 1.0)

    for b in range(BLOC):
        # ---------- phase A: load + channel pooling ----------
        xt = [[None] * NCH for _ in range(NH)]
        chmax_p = []
        chsum_p = []
        for h in range(NH):
            cmp_t = stat.tile([P, NCH], F32, name=f"chmaxp{b}{h}", tag=f"chmaxp{h}")
            csp_t = stat.tile([P, NCH], F32, name=f"chsump{b}{h}", tag=f"chsump{h}")
            chmax_p.append(cmp_t)
            chsum_p.append(csp_t)
            if "chpool" in skip:
                nc.vector.memset(cmp_t[:], 0.5)
                nc.vector.memset(csp_t[:], 0.5)
        for k in range(NCH):
            for h in range(NH):
                t = xp.tile([P, FCH], F32, name=f"x{b}{h}{k}", tag="x")
                xt[h][k] = t
                nc.sync.dma_start(
                    out=t[:],
                    in_=x_d[b, h * P:(h + 1) * P, k * FCH:(k + 1) * FCH])
                if "chpool" in skip:
                    continue
                nc.vector.tensor_reduce(
                    out=chmax_p[h][:, k:k + 1], in_=t[:],
                    axis=mybir.AxisListType.X, op=mybir.AluOpType.max)
                # in-place copy whose only purpose is the free-dim sum output
                nc.scalar.activation(
                    out=t[:], in_=t[:], func=AF.Copy,
                    accum_out=chsum_p[h][:, k:k + 1])

        # ---------- phase B: channel-attention MLP ----------
        scale_sb = []
        z_ps = mlpp.tile([R, 1], F32, name=f"zps{b}", tag="mlp")
        hvec = []
        for h in range(NH):
            cmf = stat.tile([P, 1], F32, name=f"chmaxf{b}{h}", tag=f"chmaxf{h}")
            csf = stat.tile([P, 1], F32, name=f"chsumf{b}{h}", tag=f"chsumf{h}")
            nc.vector.tensor_reduce(out=cmf[:], in_=chmax_p[h][:],
                                    axis=mybir.AxisListType.X,
                                    op=mybir.AluOpType.max)
            nc.vector.tensor_reduce(out=csf[:], in_=chsum_p[h][:],
                                    axis=mybir.AxisListType.X,
                                    op=mybir.AluOpType.add)
            hv = stat.tile([P, 1], F32, name=f"hvec{b}{h}", tag=f"hvec{h}")
            # hv = chmax + chsum/HW
            nc.scalar.activation(out=hv[:], in_=csf[:], func=AF.Identity,
                                 bias=cmf[:, 0:1], scale=1.0 / HW)
            hvec.append(hv)
        for h in range(NH):
            nc.tensor.matmul(out=z_ps[:], lhsT=w1t_sb[:, h * R:(h + 1) * R],
                             rhs=hvec[h][:], start=(h == 0), stop=(h == NH - 1))
        zr = stat.tile([R, 1], F32, name=f"zrelu{b}", tag="zrelu")
        nc.scalar.activation(out=zr[:], in_=z_ps[:], func=AF.Relu)
        for h in range(NH):
            l_ps = mlpp.tile([P, 1], F32, name=f"lps{b}{h}", tag="mlp")
            nc.tensor.matmul(out=l_ps[:], lhsT=w2t_sb[:, h * P:(h + 1) * P],
                             rhs=zr[:], start=True, stop=True)
            sc = stat.tile([P, 1], F32, name=f"scale{b}{h}", tag=f"scale{h}")
            nc.scalar.activation(out=sc[:], in_=l_ps[:], func=AF.Sigmoid)
            scale_sb.append(sc)

        # ---------- phase C/D/E: scale, spatial stats, conv, final ----------
        smaxT = stat.tile([P, H], F32, name=f"smaxT{b}", tag="smaxT")   # [w, h]
        savgT = stat.tile([P, H], F32, name=f"savgT{b}", tag="savgT")   # [w, h]
        conv_ps = convp.tile([P, H], F32, name=f"convps{b}", tag="conv")
        bcs = {}
        if "trans" in skip:
            nc.vector.memset(smaxT[:], 0.25)
        if "savg" in skip:
            nc.vector.memset(savgT[:], 0.25)

        def conv_pair(g):
            h0c, h1c = g * CONVG * HROWS, (g + 1) * CONVG * HROWS
            # 7x7 conv as banded matmuls: out[:, h] += bandT_{c,i} @ statT[:, h+i-3]
            mms = []
            for c, st in ((0, smaxT), (1, savgT)):
                for i in range(7):
                    lo = max(h0c, 3 - i)
                    hi = min(h1c, H + 3 - i)
                    if lo >= hi:
                        continue
                    mms.append((c, i, lo, hi, st))
            # identity-shift tap first so start=True covers the whole column range
            mms.sort(key=lambda m: (m[1] != 3 or m[0] != 0))
            for n, (c, i, lo, hi, st) in enumerate(mms):
                assert not (n == 0 and (lo != h0c or hi != h1c))
                nc.tensor.matmul(
                    out=conv_ps[:, lo:hi],
                    lhsT=bands_sb[:, (c * 7 + i) * P:(c * 7 + i + 1) * P],
                    rhs=st[:, lo + i - 3:hi + i - 3],
                    start=(n == 0), stop=(n == len(mms) - 1),
                    skip_group_check=True)

        def attn_chunk(kc):
            h0c, h1c = kc * HROWS, (kc + 1) * HROWS
            attn_wh = stat.tile([P, HROWS], F32, name=f"attnwh{b}{kc}",
                                tag="attnwh", bufs=3)
            nc.scalar.activation(out=attn_wh[:], in_=conv_ps[:, h0c:h1c],
                                 func=AF.Sigmoid)
            at_ps = atpp.tile([HROWS, P], F32, name=f"atps{b}{kc}", tag="atp")
            nc.tensor.transpose(out=at_ps[:], in_=attn_wh[:], identity=ident_sb[:])
            attn_hw = stat.tile([HROWS, P], F32, name=f"attnhw{b}{kc}",
                                tag="attnhw", bufs=3)
            nc.scalar.activation(out=attn_hw[:], in_=at_ps[:], func=AF.Copy)
            row = rowp.tile([1, FCH], F32, name=f"row{b}{kc}", tag="row")
            nc.sync.dma_start(
                out=row[:].rearrange("p (h w) -> p h w", h=HROWS),
                in_=attn_hw[:])
            bc = bcp.tile([P, FCH], F32, name=f"bc{b}{kc}", tag="bc")
            nc.gpsimd.partition_broadcast(bc[:], row[:], channels=P)
            bcs[kc] = bc

        def conv_and_final(g):
            if "conv" not in skip:
                conv_pair(g)
            for kc in range(CONVG * g, CONVG * (g + 1)):
                if "conv" not in skip:
                    attn_chunk(kc)
                for h in range(NH):
                    if "final" not in skip and "conv" not in skip:
                        nc.vector.tensor_mul(xt[h][kc][:], xt[h][kc][:],
                                             bcs[kc][:])
                    nc.sync.dma_start(
                        out=out_d[b, h * P:(h + 1) * P,
                                  kc * FCH:(kc + 1) * FCH],
                        in_=xt[h][kc][:])

        for k in range(NCH):
            if "scale" not in skip:
                for h in range(NH):
                    nc.scalar.activation(out=xt[h][k][:], in_=xt[h][k][:],
                                         func=AF.Copy,
                                         scale=scale_sb[h][:, 0:1])
            # spatial max over C: combine halves, transpose, segmented reduce.
            # spatial sum over C: transpose both halves into the same PSUM
            # region with accumulation, then segmented add-reduce.
            if "trans" not in skip:
                xm = xmaxp.tile([P, FCH], F32, name=f"xm{b}{k}", tag="xm")
                nc.vector.tensor_max(xm[:], xt[0][k][:], xt[1][k][:])
                for j4 in range(NBLK // 4):
                    tpt = tp.tile([P, 4 * P], F32, name=f"tp{b}{k}{j4}",
                                  tag="tp")
                    tps = tsp.tile([P, 4 * P], F32, name=f"ts{b}{k}{j4}",
                                   tag="ts")
                    for jj in range(4):
                        j = j4 * 4 + jj
                        nc.tensor.transpose(out=tpt[:, jj * P:(jj + 1) * P],
                                            in_=xm[:, j * P:(j + 1) * P],
                                            identity=ident_sb[:])
                        if "savg" in skip:
                            continue
                        nc.tensor.matmul(out=tps[:, jj * P:(jj + 1) * P],
                                         lhsT=xt[0][k][:, j * P:(j + 1) * P],
                                         rhs=ident_sb[:], is_transpose=True,
                                         start=True, stop=False,
                                         skip_group_check=True)
                        nc.tensor.matmul(out=tps[:, jj * P:(jj + 1) * P],
                                         lhsT=xt[1][k][:, j * P:(j + 1) * P],
                                         rhs=ident_sb[:], is_transpose=True,
                                         start=False, stop=True,
                                         skip_group_check=True)
                    g0 = k * NBLK + j4 * 4
                    nc.vector.tensor_reduce(
                        out=smaxT[:, g0:g0 + 4],
                        in_=tpt[:].rearrange("p (b f) -> p b f", b=4),
                        axis=mybir.AxisListType.X, op=mybir.AluOpType.max)
                    if "savg" not in skip:
                        nc.vector.tensor_reduce(
                            out=savgT[:, g0:g0 + 4],
                            in_=tps[:].rearrange("p (b f) -> p b f", b=4),
                            axis=mybir.AxisListType.X, op=mybir.AluOpType.add)
            if k >= CONVG and k % CONVG == 0:
                conv_and_final((k - CONVG) // CONVG)
        conv_and_final(NCH // CONVG - 1)


def _build_nc(reps: int = 1, skip=frozenset()):
    nc = bacc.Bacc("TRN2", target_bir_lowering=False, debug=False,
                   num_devices=NCORES)
    x_d = nc.dram_tensor("x", [BLOC, C, HW], F32, kind="ExternalInput").ap()
    w1t_d = nc.dram_tensor("w1t", [C, R], F32, kind="ExternalInput").ap()
    w2t_d = nc.dram_tensor("w2t", [R, C], F32, kind="ExternalInput").ap()
    bands_d = nc.dram_tensor("bands", [14, W, W], F32, kind="ExternalInput").ap()
    ident_d = nc.dram_tensor("ident", [P, P], F32, kind="ExternalInput").ap()
    out_d = nc.dram_tensor("out", [BLOC, C, HW], F32, kind="ExternalOutput").ap()
    with tile.TileContext(nc) as tc:
        for _ in range(reps):
            csa_kernel(tc, out_d, x_d, w1t_d, w2t_d, bands_d, ident_d,
                       skip=skip)
    nc.compile()
    return nc


_NC_CACHE = None


def _get_nc():
    global _NC_CACHE
    if _NC_CACHE is None:
        _NC_CACHE = _build_nc()
    return _NC_CACHE


def build_bands(w_conv):
    """[14, W, W] transposed band matrices; bands[c*7+i][w', w] =
    w_conv[0, c, i, w'-w+3]; avg channel folded with 1/C."""
    w_conv = np.asarray(w_conv, np.float32)
    bands = np.zeros((2, 7, W, W), np.float32)
    for c in range(2):
        for i in range(7):
            for kj in range(7):
                bands[c, i] += w_conv[0, c, i, kj] * np.eye(W, k=3 - kj,
                                                            dtype=np.float32)
    bands[1] /= C
    return bands.reshape(14, W, W)


def make_in_maps(x, w_fc1, w_fc2, w_conv):
    x = np.ascontiguousarray(np.asarray(x, np.float32))
    w1t = np.ascontiguousarray(np.asarray(w_fc1, np.float32).T)
    w2t = np.ascontiguousarray(np.asarray(w_fc2, np.float32).T)
    bands = build_bands(w_conv)
    ident = np.eye(P, dtype=np.float32)
    xr = x.reshape(NCORES, BLOC, C, HW)
    return [{"x": np.ascontiguousarray(xr[i]), "w1t": w1t, "w2t": w2t,
             "bands": bands, "ident": ident} for i in range(NCORES)]


def kernel(x, w_fc1, w_fc2, w_conv):
    nc = _get_nc()
    in_maps = make_in_maps(x, w_fc1, w_fc2, w_conv)
    res = run_bass_kernel_spmd(nc, in_maps, list(range(NCORES)))
    out = np.stack([res.results[i]["out"] for i in range(NCORES)])
    return out.reshape(B, C, H, W).astype(np.float32)



# revision 18
# speedup vs baseline: 1.0265x; 1.0265x over previous
"""CombinedCSA (channel+spatial attention) Trainium2 Bass kernel, v2.

Sharding: data-parallel over batch. 16 images / 8 cores = 2 images per core.
Weights (fc1/fc2/conv bands) replicated, pre-transposed host-side.

v2 dataflow (fp16 compute domain, conv in [h,w] layout, no PE transposes):
  A: load fp32 chunk -> ACT copy->fp16 (+accum_out = channel sum)
     -> DVE running channel max (fp16 TT, 2x mode)
  B: MLP (PE fp32 matmuls + ACT relu/sigmoid) -> per-channel scale [128,1]
  C: DVE in-place scale (tensor_scalar, 4x mode); TT max of halves;
     gpsimd partition reduce (axis=C, max) -> smax rows;
     PE ones-stationary matmuls -> spatial sum in PSUM -> DMA to stat tile
  D: conv = 14 banded matmuls in [h,w] layout (bands over h built host-side,
     w handled by shifted rhs columns); sigmoid -> attn fp16
  E: gpsimd partition_broadcast of attn row; DVE TT multiply in place;
     DMA store fp16 (host upcasts to fp32)
"""

import numpy as np
from contextlib import ExitStack

import concourse.bass as bass
import concourse.bass_isa as bass_isa
import concourse.tile as tile
from concourse import bacc, mybir
from concourse._compat import with_exitstack
from concourse.bass_utils import run_bass_kernel_spmd

F32 = mybir.dt.float32
F16 = mybir.dt.float16
AF = mybir.ActivationFunctionType
ALU = mybir.AluOpType
AX = mybir.AxisListType

# Problem constants (hardcoded; see spec)
B, C, H, W = 16, 256, 128, 128
HW = H * W          # 16384
R = 16              # Cr = C // 16
NCORES = 8
BLOC = B // NCORES  # 2 images per core
NH = 2              # channel halves of 128
P = 128
FCH = 2048          # hw elements per chunk (16 h-rows)
NCH = HW // FCH     # 8 chunks per image
HROWS = FCH // W    # 16 h-rows per chunk
NBLK = FCH // 512   # 4 psum blocks of 512 per chunk


@with_exitstack
def csa_kernel(ctx, tc, out_d, x_d, w1t_d, w2t_d, b16_d,
               skip=frozenset()):
    nc = tc.nc

    # ---- pools ----
    xp = ctx.enter_context(tc.tile_pool(name="xp", bufs=3))        # fp32 stage
    xbp = ctx.enter_context(tc.tile_pool(name="xbp", bufs=28))     # fp16 image
    tmp = ctx.enter_context(tc.tile_pool(name="tmp", bufs=2))      # max scratch
    bcp = ctx.enter_context(tc.tile_pool(name="bcp", bufs=2))      # attn bcast
    arp = ctx.enter_context(tc.tile_pool(name="arp", bufs=2))      # allred out
    arow = ctx.enter_context(tc.tile_pool(name="arow", bufs=2))    # attn rows
    rmp = ctx.enter_context(tc.tile_pool(name="rmp", bufs=2))      # run max
    stat = ctx.enter_context(tc.tile_pool(name="stat", bufs=2))
    smallp = ctx.enter_context(tc.tile_pool(name="small", bufs=2))
    cons = ctx.enter_context(tc.tile_pool(name="cons", bufs=1))
    svp = ctx.enter_context(tc.tile_pool(name="svp", bufs=4, space="PSUM"))
    convp = ctx.enter_context(tc.tile_pool(name="convp", bufs=2, space="PSUM"))
    mlpp = ctx.enter_context(tc.tile_pool(name="mlpp", bufs=1, space="PSUM"))

    # ---- constants / weights ----
    w1t_sb = cons.tile([P, NH * R], F32)           # col block h = w_fc1.T half h
    for h in range(NH):
        nc.sync.dma_start(out=w1t_sb[:, h * R:(h + 1) * R],
                          in_=w1t_d[h * P:(h + 1) * P, :])
    w2t_sb = cons.tile([R, C], F32)                # [16, 256] = w_fc2.T
    nc.sync.dma_start(out=w2t_sb[:], in_=w2t_d[:])
    b16_sb = cons.tile([P, 14 * P], F16)           # conv bands, [h, cj*128+h']
    nc.sync.dma_start(out=b16_sb[:].rearrange("p (j f) -> p j f", j=14),
                      in_=b16_d.transpose([1, 0, 2]))
    ones16 = cons.tile([P, 1], F16)
    nc.vector.memset(ones16[:], 1.0)

    for b in range(BLOC):
        # ---------- phase A: load + convert + channel pooling ----------
        xb = [[None] * NCH for _ in range(NH)]
        chsum_p = []
        rm = []
        for h in range(NH):
            csp_t = smallp.tile([P, NCH], F32, name=f"chsump{b}{h}",
                                tag=f"chsump{h}")
            chsum_p.append(csp_t)
            rm.append(rmp.tile([P, FCH], F16, name=f"rm{b}{h}", tag=f"rm{h}"))
            if "chpool" in skip:
                nc.vector.memset(csp_t[:], 0.5)
        for k in range(NCH):
            for h in range(NH):
                xf = xp.tile([P, FCH], F32, name=f"xf{b}{h}{k}", tag="xf")
                nc.sync.dma_start(
                    out=xf[:],
                    in_=x_d[b, h * P:(h + 1) * P, k * FCH:(k + 1) * FCH])
                t = xbp.tile([P, FCH], F16, name=f"xb{b}{h}{k}", tag="xb")
                xb[h][k] = t
                if "chpool" in skip:
                    nc.scalar.activation(out=t[:], in_=xf[:], func=AF.Copy)
                    continue
                nc.scalar.activation(out=t[:], in_=xf[:], func=AF.Copy,
                                     accum_out=chsum_p[h][:, k:k + 1])
                if k == 0:
                    nc.vector.tensor_copy(rm[h][:], t[:])
                else:
                    nc.vector.tensor_max(rm[h][:], rm[h][:], t[:])

        # ---------- phase B: channel-attention MLP ----------
        sc = []
        z_ps = mlpp.tile([R, 1], F32, name=f"zps{b}", tag="mlp")
        hvec = []
        for h in range(NH):
            cmf = smallp.tile([P, 1], F32, name=f"chmaxf{b}{h}", tag=f"cmf{h}")
            csf = smallp.tile([P, 1], F32, name=f"chsumf{b}{h}", tag=f"csf{h}")
            if "chpool" in skip:
                nc.vector.memset(cmf[:], 0.5)
                nc.vector.memset(csf[:], 0.5)
            else:
                nc.vector.tensor_reduce(out=cmf[:], in_=rm[h][:],
                                        axis=AX.X, op=ALU.max)
                nc.vector.tensor_reduce(out=csf[:], in_=chsum_p[h][:],
                                        axis=AX.X, op=ALU.add)
            hv = smallp.tile([P, 1], F32, name=f"hvec{b}{h}", tag=f"hvec{h}")
            # hv = chmax + chsum/HW
            nc.scalar.activation(out=hv[:], in_=csf[:], func=AF.Identity,
                                 bias=cmf[:, 0:1], scale=1.0 / HW)
            hvec.append(hv)
        for h in range(NH):
            nc.tensor.matmul(out=z_ps[:], lhsT=w1t_sb[:, h * R:(h + 1) * R],
                             rhs=hvec[h][:], start=(h == 0), stop=(h == NH - 1))
        zr = smallp.tile([R, 1], F32, name=f"zrelu{b}", tag="zrelu")
        nc.scalar.activation(out=zr[:], in_=z_ps[:], func=AF.Relu)
        for h in range(NH):
            l_ps = mlpp.tile([P, 1], F32, name=f"lps{b}{h}", tag="mlp")
            nc.tensor.matmul(out=l_ps[:], lhsT=w2t_sb[:, h * P:(h + 1) * P],
                             rhs=zr[:], start=True, stop=True)
            s = smallp.tile([P, 1], F32, name=f"scale{b}{h}", tag=f"scale{h}")
            nc.scalar.activation(out=s[:], in_=l_ps[:], func=AF.Sigmoid)
            sc.append(s)

        # ---------- phase C: scale in place + spatial stats ----------
        savg_st = stat.tile([P, W], F16, name=f"savgst{b}", tag="savgst")
        smax_st = stat.tile([P, W], F16, name=f"smaxst{b}", tag="smaxst")
        for k in range(NCH):
            if "scale" not in skip:
                for h in range(NH):
                    nc.vector.tensor_scalar_mul(xb[h][k][:], xb[h][k][:],
                                                sc[h][:, 0:1])
            if "smax" not in skip:
                tm = tmp.tile([P, FCH], F16, name=f"tm{b}{k}", tag="tm")
                nc.vector.tensor_max(tm[:], xb[0][k][:], xb[1][k][:])
                ar = arp.tile([P, FCH], F16, name=f"ar{b}{k}", tag="ar")
                nc.gpsimd.partition_all_reduce(
                    ar[:], tm[:], channels=P,
                    reduce_op=bass_isa.ReduceOp.max)
                nc.sync.dma_start(
                    out=smax_st[k * HROWS:(k + 1) * HROWS, :],
                    in_=ar[0:1, :].rearrange("p (r w) -> p r w", r=HROWS))
            if "savg" not in skip:
                svrow = arow.tile([1, FCH], F16, name=f"svrow{b}{k}",
                                  tag="svrow")
                for j in range(NBLK):
                    sv_ps = svp.tile([1, 512], F32, name=f"svps{b}{k}{j}",
                                     tag="sv", bufs=4)
                    for h in range(NH):
                        nc.tensor.matmul(
                            out=sv_ps[:], lhsT=ones16[:],
                            rhs=xb[h][k][:, j * 512:(j + 1) * 512],
                            start=(h == 0), stop=(h == NH - 1))
                    nc.scalar.activation(
                        out=svrow[0:1, j * 512:(j + 1) * 512],
                        in_=sv_ps[:], func=AF.Copy)
                nc.sync.dma_start(
                    out=savg_st[k * HROWS:(k + 1) * HROWS, :],
                    in_=svrow[:].rearrange("p (r w) -> p r w", r=HROWS))

        # ---------- phase D: conv + sigmoid ----------
        attn16 = stat.tile([P, W], F16, name=f"attn{b}", tag="attn")
        if "conv" not in skip:
            if "smax" in skip:
                nc.vector.memset(smax_st[:], 0.25)
            if "savg" in skip:
                nc.vector.memset(savg_st[:], 0.25)
            conv_ps = convp.tile([P, W], F32, name=f"convps{b}", tag="conv")
            # out[h',w'] += sum_h band[c,j][h,h'] * stat_c[h, w'+j-3]
            mms = []
            for cc, st_t in ((0, smax_st), (1, savg_st)):
                for j in range(7):
                    lo = max(0, 3 - j)
                    hi = min(W, W + 3 - j)
                    mms.append((cc, j, lo, hi, st_t))
            mms.sort(key=lambda m: (m[1] != 3 or m[0] != 0))
            for n, (cc, j, lo, hi, st_t) in enumerate(mms):
                nc.tensor.matmul(
                    out=conv_ps[:, lo:hi],
                    lhsT=b16_sb[:, (cc * 7 + j) * P:(cc * 7 + j + 1) * P],
                    rhs=st_t[:, lo + j - 3:hi + j - 3],
                    start=(n == 0), stop=(n == len(mms) - 1),
                    skip_group_check=True)
            nc.scalar.activation(out=attn16[:], in_=conv_ps[:], func=AF.Sigmoid)
        else:
            nc.vector.memset(attn16[:], 0.5)

        # ---------- phase E: broadcast + final multiply + store ----------
        for k in range(NCH):
            if "final" not in skip:
                arow_t = arow.tile([1, FCH], F16, name=f"arow{b}{k}",
                                   tag="arow", bufs=3)
                nc.sync.dma_start(
                    out=arow_t[:].rearrange("p (r w) -> p r w", r=HROWS),
                    in_=attn16[k * HROWS:(k + 1) * HROWS, :])
                bc = bcp.tile([P, FCH], F16, name=f"bc{b}{k}", tag="bc")
                nc.gpsimd.partition_broadcast(bc[:], arow_t[:], channels=P)
                for h in range(NH):
                    nc.vector.tensor_mul(xb[h][k][:], xb[h][k][:], bc[:])
            for h in range(NH):
                nc.sync.dma_start(
                    out=out_d[b, h * P:(h + 1) * P, k * FCH:(k + 1) * FCH],
                    in_=xb[h][k][:])


def _build_nc(reps: int = 1, skip=frozenset()):
    nc = bacc.Bacc("TRN2", target_bir_lowering=False, debug=False,
                   num_devices=NCORES)
    x_d = nc.dram_tensor("x", [BLOC, C, HW], F32, kind="ExternalInput").ap()
    w1t_d = nc.dram_tensor("w1t", [C, R], F32, kind="ExternalInput").ap()
    w2t_d = nc.dram_tensor("w2t", [R, C], F32, kind="ExternalInput").ap()
    b16_d = nc.dram_tensor("b16", [14, H, H], F16, kind="ExternalInput").ap()
    out_d = nc.dram_tensor("out", [BLOC, C, HW], F16, kind="ExternalOutput").ap()
    with tile.TileContext(nc) as tc:
        for _ in range(reps):
            csa_kernel(tc, out_d, x_d, w1t_d, w2t_d, b16_d, skip=skip)
    nc.compile()
    return nc


_NC_CACHE = None


def _get_nc():
    global _NC_CACHE
    if _NC_CACHE is None:
        _NC_CACHE = _build_nc()
    return _NC_CACHE


def build_bands(w_conv):
    """[14, H, H] band matrices for [h,w]-layout conv.

    bands[c*7+j][h_in, h_out] = w_conv[0, c, h_in-h_out+3, j]; the avg
    channel (c=1) is folded with 1/C."""
    w_conv = np.asarray(w_conv, np.float32)
    bands = np.zeros((2, 7, H, H), np.float32)
    for c in range(2):
        for j in range(7):
            for ki in range(7):
                bands[c, j] += w_conv[0, c, ki, j] * np.eye(H, k=3 - ki,
                                                            dtype=np.float32)
    bands[1] /= C
    return bands.reshape(14, H, H)


def make_in_maps(x, w_fc1, w_fc2, w_conv):
    x = np.ascontiguousarray(np.asarray(x, np.float32))
    w1t = np.ascontiguousarray(np.asarray(w_fc1, np.float32).T)
    w2t = np.ascontiguousarray(np.asarray(w_fc2, np.float32).T)
    b16 = build_bands(w_conv).astype(np.float16)
    xr = x.reshape(NCORES, BLOC, C, HW)
    return [{"x": np.ascontiguousarray(xr[i]), "w1t": w1t, "w2t": w2t,
             "b16": b16} for i in range(NCORES)]


def kernel(x, w_fc1, w_fc2, w_conv):
    nc = _get_nc()
    in_maps = make_in_maps(x, w_fc1, w_fc2, w_conv)
    res = run_bass_kernel_spmd(nc, in_maps, list(range(NCORES)))
    out = np.stack([np.asarray(res.results[i]["out"], dtype=np.float32)
                    for i in range(NCORES)])
    return out.reshape(B, C, H, W)


# revision 44
# speedup vs baseline: 1.3622x; 1.3270x over previous
"""CombinedCSA (channel+spatial attention) Trainium2 Bass kernel, v2.

Sharding: data-parallel over batch. 16 images / 8 cores = 2 images per core.
Weights (fc1/fc2/conv bands) replicated, pre-transposed host-side.

v2 dataflow (fp16 compute domain, conv in [h,w] layout, no PE transposes):
  A: load fp32 chunk -> ACT copy->fp16 (+accum_out = channel sum)
     -> DVE running channel max (fp16 TT, 2x mode)
  B: MLP (PE fp32 matmuls + ACT relu/sigmoid) -> per-channel scale [128,1]
  C: DVE in-place scale (tensor_scalar, 4x mode); TT max of halves;
     gpsimd partition_all_reduce(max) -> smax rows;
     PE ones-stationary matmuls -> spatial sum in PSUM -> ACT copy + DMA
  D: conv = 14 banded matmuls in [h,w] layout (bands over h built host-side,
     w handled by shifted rhs columns); sigmoid -> attn fp16
  E: gpsimd partition_broadcast of attn row; DVE TT multiply in place;
     DMA store fp16 (host upcasts to fp32)

Emission is software-pipelined across the two images so every engine's
in-order stream and the SP DMA queue interleave image 1's loads with
image 0's compute:
  A(0) B(0) [C(0,k) | A(1,k)] D(0) E(0) B(1) C(1) D(1) E(1)
DMA queues: loads + small rows on SP HWDGE, stores on ACT HWDGE.
"""

import os
import numpy as np
from contextlib import ExitStack

import concourse.bass as bass
import concourse.bass_isa as bass_isa
import concourse.tile as tile
from concourse import bacc, mybir
from concourse._compat import with_exitstack
from concourse.bass_utils import run_bass_kernel_spmd

F32 = mybir.dt.float32
F16 = mybir.dt.float16
AF = mybir.ActivationFunctionType
ALU = mybir.AluOpType
AX = mybir.AxisListType

# Problem constants (hardcoded; see spec)
B, C, H, W = 16, 256, 128, 128
HW = H * W          # 16384
R = 16              # Cr = C // 16
NCORES = 8
BLOC = B // NCORES  # 2 images per core
NH = 2              # channel halves of 128
P = 128
FCH = 2048          # hw elements per chunk (16 h-rows)
NCH = HW // FCH     # 8 chunks per image
HROWS = FCH // W    # 16 h-rows per chunk
NBLK = FCH // 512   # 4 psum blocks of 512 per chunk
CHUNKED = os.environ.get("CSA_CHUNKED", "1") == "1"


class _Img:
    """Per-image tile state."""
    def __init__(self):
        self.xb = [[None] * NCH for _ in range(NH)]
        self.chsum = None
        self.rm = None
        self.sc = None
        self.savg_st = None
        self.smax_st = None
        self.attn16 = [None] * 2


@with_exitstack
def csa_kernel(ctx, tc, out_d, x_d, w1t_d, w2t_d, b16_d,
               skip=frozenset()):
    nc = tc.nc

    # ---- pools ----
    xp = ctx.enter_context(tc.tile_pool(name="xp", bufs=3))        # fp32 stage
    xbp = ctx.enter_context(tc.tile_pool(name="xbp", bufs=27))     # fp16 image
    tmp = ctx.enter_context(tc.tile_pool(name="tmp", bufs=2))      # max scratch
    bcp = ctx.enter_context(tc.tile_pool(name="bcp", bufs=3))      # attn bcast
    arp = ctx.enter_context(tc.tile_pool(name="arp", bufs=3))      # allred out
    arow = ctx.enter_context(tc.tile_pool(name="arow", bufs=2))    # rows
    rmp = ctx.enter_context(tc.tile_pool(name="rmp", bufs=2))      # run max
    stat = ctx.enter_context(tc.tile_pool(name="stat", bufs=2))
    smallp = ctx.enter_context(tc.tile_pool(name="small", bufs=2))
    cons = ctx.enter_context(tc.tile_pool(name="cons", bufs=1))
    svp = ctx.enter_context(tc.tile_pool(name="svp", bufs=4, space="PSUM"))
    convp = ctx.enter_context(tc.tile_pool(name="convp", bufs=2, space="PSUM"))
    mlpp = ctx.enter_context(tc.tile_pool(name="mlpp", bufs=1, space="PSUM"))

    # ---- constants / weights ----
    w1t_sb = cons.tile([P, NH * R], F32)           # col block h = w_fc1.T half h
    for h in range(NH):
        nc.sync.dma_start(out=w1t_sb[:, h * R:(h + 1) * R],
                          in_=w1t_d[h * P:(h + 1) * P, :])
    w2t_sb = cons.tile([R, C], F32)                # [16, 256] = w_fc2.T
    nc.sync.dma_start(out=w2t_sb[:], in_=w2t_d[:])
    b16_sb = cons.tile([P, 14 * P], F16)           # conv bands, [h, cj*128+h']
    nc.sync.dma_start(out=b16_sb[:].rearrange("p (j f) -> p j f", j=14),
                      in_=b16_d.transpose([1, 0, 2]))
    ones16 = cons.tile([P, 1], F16)
    nc.vector.memset(ones16[:], 1.0)

    st = [_Img() for _ in range(BLOC)]

    def emit_A_init(b):
        im = st[b]
        im.chsum = []
        im.rm = []
        for h in range(NH):
            csp_t = smallp.tile([P, NCH], F32, name=f"chsump{b}{h}",
                                tag=f"chsump{h}")
            im.chsum.append(csp_t)
            im.rm.append(rmp.tile([P, 512], F16, name=f"rm{b}{h}",
                                  tag=f"rm{h}"))
            if "chpool" in skip:
                nc.vector.memset(csp_t[:], 0.5)

    def emit_A(b, k):
        im = st[b]
        for h in range(NH):
            xf = xp.tile([P, FCH], F32, name=f"xf{b}{h}{k}", tag="xf")
            nc.sync.dma_start(
                out=xf[:],
                in_=x_d[b, h * P:(h + 1) * P, k * FCH:(k + 1) * FCH])
            t = xbp.tile([P, FCH], F16, name=f"xb{b}{h}{k}", tag="xb")
            im.xb[h][k] = t
            if "chpool" in skip:
                nc.scalar.activation(out=t[:], in_=xf[:], func=AF.Copy)
                continue
            nc.scalar.activation(out=t[:], in_=xf[:], func=AF.Copy,
                                 accum_out=im.chsum[h][:, k:k + 1])
            # fold chunk to width 512, then merge into the running max
            f1 = tmp.tile([P, 1024], F16, name=f"f1{b}{h}{k}", tag="fold",
                          bufs=2)
            nc.vector.tensor_max(f1[:], t[:, 0:1024], t[:, 1024:2048])
            if k == 0:
                nc.vector.tensor_max(im.rm[h][:], f1[:, 0:512],
                                     f1[:, 512:1024])
            else:
                nc.vector.tensor_max(im.rm[h][:], im.rm[h][:], f1[:, 0:512])
                nc.vector.tensor_max(im.rm[h][:], im.rm[h][:],
                                     f1[:, 512:1024])

    def emit_B(b):
        im = st[b]
        im.sc = []
        z_ps = mlpp.tile([R, 1], F32, name=f"zps{b}", tag="mlp")
        hvec = []
        for h in range(NH):
            cmf = smallp.tile([P, 1], F32, name=f"chmaxf{b}{h}", tag=f"cmf{h}")
            csf = smallp.tile([P, 1], F32, name=f"chsumf{b}{h}", tag=f"csf{h}")
            if "chpool" in skip:
                nc.vector.memset(cmf[:], 0.5)
                nc.vector.memset(csf[:], 0.5)
            else:
                nc.vector.tensor_reduce(out=cmf[:], in_=im.rm[h][:],
                                        axis=AX.X, op=ALU.max)
                nc.vector.tensor_reduce(out=csf[:], in_=im.chsum[h][:],
                                        axis=AX.X, op=ALU.add)
            hv = smallp.tile([P, 1], F32, name=f"hvec{b}{h}", tag=f"hvec{h}")
            # hv = chmax + chsum/HW
            nc.scalar.activation(out=hv[:], in_=csf[:], func=AF.Identity,
                                 bias=cmf[:, 0:1], scale=1.0 / HW)
            hvec.append(hv)
        for h in range(NH):
            nc.tensor.matmul(out=z_ps[:], lhsT=w1t_sb[:, h * R:(h + 1) * R],
                             rhs=hvec[h][:], start=(h == 0), stop=(h == NH - 1))
        zr = smallp.tile([R, 1], F32, name=f"zrelu{b}", tag="zrelu")
        nc.scalar.activation(out=zr[:], in_=z_ps[:], func=AF.Relu)
        for h in range(NH):
            l_ps = mlpp.tile([P, 1], F32, name=f"lps{b}{h}", tag="mlp")
            nc.tensor.matmul(out=l_ps[:], lhsT=w2t_sb[:, h * P:(h + 1) * P],
                             rhs=zr[:], start=True, stop=True)
            s = smallp.tile([P, 1], F32, name=f"scale{b}{h}", tag=f"scale{h}")
            nc.scalar.activation(out=s[:], in_=l_ps[:], func=AF.Sigmoid)
            im.sc.append(s)
        im.savg_st = stat.tile([P, W], F16, name=f"savgst{b}", tag="savgst")
        im.smax_st = stat.tile([P, W], F16, name=f"smaxst{b}", tag="smaxst")

    def emit_C(b, k):
        im = st[b]
        if "scale" not in skip:
            for h in range(NH):
                nc.vector.tensor_scalar_mul(im.xb[h][k][:], im.xb[h][k][:],
                                            im.sc[h][:, 0:1])
        if "smax" not in skip:
            tm = tmp.tile([P, FCH], F16, name=f"tm{b}{k}", tag="tm")
            nc.vector.tensor_max(tm[:], im.xb[0][k][:], im.xb[1][k][:])
            ar = arp.tile([P, FCH], F16, name=f"ar{b}{k}", tag="ar")
            nc.gpsimd.partition_all_reduce(
                ar[:], tm[:], channels=P, reduce_op=bass_isa.ReduceOp.max)
            nc.sync.dma_start(
                out=im.smax_st[k * HROWS:(k + 1) * HROWS, :],
                in_=ar[0:1, :].rearrange("p (r w) -> p r w", r=HROWS))
        if "savg" not in skip:
            svrow = arow.tile([1, FCH], F16, name=f"svrow{b}{k}", tag="svrow")
            for j in range(NBLK):
                sv_ps = svp.tile([1, 512], F32, name=f"svps{b}{k}{j}",
                                 tag="sv", bufs=4)
                for h in range(NH):
                    nc.tensor.matmul(
                        out=sv_ps[:], lhsT=ones16[:],
                        rhs=im.xb[h][k][:, j * 512:(j + 1) * 512],
                        start=(h == 0), stop=(h == NH - 1))
                nc.scalar.activation(
                    out=svrow[0:1, j * 512:(j + 1) * 512],
                    in_=sv_ps[:], func=AF.Copy)
            nc.sync.dma_start(
                out=im.savg_st[k * HROWS:(k + 1) * HROWS, :],
                in_=svrow[:].rearrange("p (r w) -> p r w", r=HROWS))

    def emit_D(b, g):
        """Partial conv over an h'-half [64g, 64g+64). Contraction pieces
        stay at 32/64-aligned bases {0,32,64} with K in {3,32,64} so the
        default matmul tile_position path handles them, and each half only
        depends on stat rows written by chunks <= 4g+4."""
        im = st[b]
        if not CHUNKED:
            if g < 1:
                return
            af = stat.tile([P, W], F16, name=f"attnf{b}", tag="attnf")
            im.attn16 = [af, af]
            conv_ps = convp.tile([P, W], F32, name=f"convps{b}", tag="convf",
                                 bufs=2)
            mms = []
            for cc, st_t in ((0, im.smax_st), (1, im.savg_st)):
                for j in range(7):
                    lo = max(0, 3 - j)
                    hi = min(W, W + 3 - j)
                    mms.append((cc, j, lo, hi, st_t))
            mms.sort(key=lambda m: (m[1] != 3 or m[0] != 0))
            for n, (cc, j, lo, hi, st_t) in enumerate(mms):
                nc.tensor.matmul(
                    out=conv_ps[:, lo:hi],
                    lhsT=b16_sb[:, (cc * 7 + j) * P:(cc * 7 + j + 1) * P],
                    rhs=st_t[:, lo + j - 3:hi + j - 3],
                    start=(n == 0), stop=(n == len(mms) - 1),
                    skip_group_check=True)
            nc.scalar.activation(out=af[:], in_=conv_ps[:], func=AF.Sigmoid)
            return
        ag = stat.tile([64, W], F16, name=f"attn{b}{g}", tag="attn", bufs=4)
        im.attn16[g] = ag
        if g == 0:
            if "smax" in skip:
                nc.vector.memset(im.smax_st[:], 0.25)
            if "savg" in skip:
                nc.vector.memset(im.savg_st[:], 0.25)
        cg_ps = convp.tile([64, W], F32, name=f"convps{b}{g}", tag="conv")
        if g == 0:
            pieces = [(0, 64), (64, 3)]      # main rows + upper halo
        else:
            pieces = [(32, 32), (64, 64)]    # lower halo rows + main rows
        # out[h',w'] += sum_h band[c,j][h,h'] * stat_c[h, w'+j-3]
        mms = []
        for cc, st_t in ((0, im.smax_st), (1, im.savg_st)):
            for j in range(7):
                lo = max(0, 3 - j)
                hi = min(W, W + 3 - j)
                for pb, pk in pieces:
                    main = (pb == 64 * g)
                    mms.append((cc, j, lo, hi, st_t, pb, pk, main))
        mms.sort(key=lambda m: (m[1] != 3 or m[0] != 0 or not m[7]))
        for n, (cc, j, lo, hi, st_t, pb, pk, main) in enumerate(mms):
            nc.tensor.matmul(
                out=cg_ps[:, lo:hi],
                lhsT=b16_sb[pb:pb + pk,
                            (cc * 7 + j) * P + 64 * g:(cc * 7 + j) * P + 64 * g + 64],
                rhs=st_t[pb:pb + pk, lo + j - 3:hi + j - 3],
                start=(n == 0), stop=(n == len(mms) - 1),
                skip_group_check=True)
        nc.scalar.activation(out=ag[:], in_=cg_ps[:], func=AF.Sigmoid)

    def emit_E(b, k):
        im = st[b]
        if "final" not in skip:
            ag = im.attn16[k // 4 if CHUNKED else 0]
            r0 = ((k % 4) if CHUNKED else k) * HROWS
            arow_t = arow.tile([1, FCH], F16, name=f"arow{b}{k}",
                               tag="arow", bufs=3)
            nc.gpsimd.dma_start(
                out=arow_t[:].rearrange("p (r w) -> p r w", r=HROWS),
                in_=ag[r0:r0 + HROWS, :])
            bc = bcp.tile([P, FCH], F16, name=f"bc{b}{k}", tag="bc")
            nc.gpsimd.partition_broadcast(bc[:], arow_t[:], channels=P)
            for h in range(NH):
                nc.vector.tensor_mul(im.xb[h][k][:], im.xb[h][k][:], bc[:])
        for h in range(NH):
            nc.scalar.dma_start(
                out=out_d[b, h * P:(h + 1) * P, k * FCH:(k + 1) * FCH],
                in_=im.xb[h][k][:])

    # ---- software-pipelined emission over the two images ----
    emit_A_init(0)
    for k in range(NCH):
        emit_A(0, k)
    emit_B(0)
    emit_A_init(1)
    if CHUNKED:
        for k in range(NCH):
            emit_C(0, k)
            emit_A(1, k)
            if k == 4:
                emit_D(0, 0)
                emit_E(0, 0)
                emit_E(0, 1)
            elif k == 5:
                emit_E(0, 2)
            elif k == 6:
                emit_E(0, 3)
        emit_B(1)
        emit_D(0, 1)
        emit_E(0, 4)
        emit_C(1, 0)
        emit_E(0, 5)
        emit_C(1, 1)
        emit_E(0, 6)
        emit_C(1, 2)
        emit_E(0, 7)
        emit_C(1, 3)
        for k in range(4, NCH):
            emit_C(1, k)
            if k == 4:
                emit_D(1, 0)
                emit_E(1, 0)
                emit_E(1, 1)
            elif k == 5:
                emit_E(1, 2)
            elif k == 6:
                emit_E(1, 3)
        emit_D(1, 1)
        for k in range(4, NCH):
            emit_E(1, k)
    else:
        for k in range(NCH):
            emit_C(0, k)
            emit_A(1, k)
        emit_D(0, 3)
        emit_B(1)
        for k in range(NCH):
            emit_E(0, k)
            emit_C(1, k)
        emit_D(1, 3)
        for k in range(NCH):
            emit_E(1, k)


def _build_nc(reps: int = 1, skip=frozenset()):
    nc = bacc.Bacc("TRN2", target_bir_lowering=False, debug=False,
                   num_devices=NCORES)
    x_d = nc.dram_tensor("x", [BLOC, C, HW], F32, kind="ExternalInput").ap()
    w1t_d = nc.dram_tensor("w1t", [C, R], F32, kind="ExternalInput").ap()
    w2t_d = nc.dram_tensor("w2t", [R, C], F32, kind="ExternalInput").ap()
    b16_d = nc.dram_tensor("b16", [14, H, H], F16, kind="ExternalInput").ap()
    out_d = nc.dram_tensor("out", [BLOC, C, HW], F16, kind="ExternalOutput").ap()
    with tile.TileContext(nc) as tc:
        for _ in range(reps):
            csa_kernel(tc, out_d, x_d, w1t_d, w2t_d, b16_d, skip=skip)
    nc.compile()
    return nc


_NC_CACHE = None


def _get_nc():
    global _NC_CACHE
    if _NC_CACHE is None:
        _NC_CACHE = _build_nc()
    return _NC_CACHE


def build_bands(w_conv):
    """[14, H, H] band matrices for [h,w]-layout conv.

    bands[c*7+j][h_in, h_out] = w_conv[0, c, h_in-h_out+3, j]; the avg
    channel (c=1) is folded with 1/C."""
    w_conv = np.asarray(w_conv, np.float32)
    bands = np.zeros((2, 7, H, H), np.float32)
    for c in range(2):
        for j in range(7):
            for ki in range(7):
                bands[c, j] += w_conv[0, c, ki, j] * np.eye(H, k=3 - ki,
                                                            dtype=np.float32)
    bands[1] /= C
    return bands.reshape(14, H, H)


def make_in_maps(x, w_fc1, w_fc2, w_conv):
    x = np.ascontiguousarray(np.asarray(x, np.float32))
    w1t = np.ascontiguousarray(np.asarray(w_fc1, np.float32).T)
    w2t = np.ascontiguousarray(np.asarray(w_fc2, np.float32).T)
    b16 = build_bands(w_conv).astype(np.float16)
    xr = x.reshape(NCORES, BLOC, C, HW)
    return [{"x": np.ascontiguousarray(xr[i]), "w1t": w1t, "w2t": w2t,
             "b16": b16} for i in range(NCORES)]


def kernel(x, w_fc1, w_fc2, w_conv):
    nc = _get_nc()
    in_maps = make_in_maps(x, w_fc1, w_fc2, w_conv)
    res = run_bass_kernel_spmd(nc, in_maps, list(range(NCORES)))
    out = np.stack([np.asarray(res.results[i]["out"], dtype=np.float32)
                    for i in range(NCORES)])
    return out.reshape(B, C, H, W)
